# revision 24
# baseline (speedup 1.0000x reference)
"""Trainium2 Bass kernel for nn_Attention (B=4, S=1024, D=1024, H=16).

Sharding: 8 cores = 4 batches x 2 query-halves. Core i handles batch i//2,
query rows [(i%2)*512, (i%2)*512+512). Each core computes the full K/V
projections for its batch (duplicated across the 2 cores sharing a batch),
all 16 heads of attention for its query slice, and the output projection.
No collectives; the output is gathered host-side.

Device dataflow (per core) — fp16 matmul operands, fp32 PSUM accumulation:
  - host passes pre-transposed qT [D,SQ], kT/vT [D,SK], W{v,o}.T [D,D], and
    m-blocked W{k,q}.T [NT,D,128] (PE contracts over the partition dim, so
    both matmul operands need the contraction dim on partitions; transposing
    and blocking on host costs nothing on HW)
  - qhT[o,sq] = (Wq.T*SCALE).T-tiles @ qT     (o on partitions)
  - khT[o,sk] likewise; vh[sk, h, dh] natural via vT-as-stationary
  - scoresT[sk,sq] per head = khT-tile.T @ qhT; the two heads of a pair run
    as K=64 matmuls packed at PE row strips 0:64 / 64:128, writing the two
    halves of one [128, 2*SQ] PSUM tile -> ONE fused exp per pair
  - expT = exp(scoresT) on ACT (no max subtraction: |scores| < ~4 here,
    and softmax(x) == softmax(x - max) exactly)
  - ctxT_aug[dh+1, sq] += [vh | 1].T @ expT  (ones column makes row 64 the
    softmax denominator, riding free on the ctx matmul)
  - ctx PSUM is drained fast (ACT copies + approx-reciprocal of the sum
    row); the normalization multiply runs later, off the critical path
  - out[sq,o] = ctxT-tiles.T @ Wo.T + bo      (natural layout -> direct DMA)

The attention phase is ACT(exp)-bound per-step, so projection-matmul
"filler" groups are interleaved into the attention emission order (with
need-driven draining) to keep the PE busy and HAM-warm throughout.

Bias handling (exact): bq via per-partition add on the qh copy; bk dropped
(softmax is invariant to per-query score shifts); bv added after normalize
(softmax rows sum to 1); bo via a partition-broadcast tile.
"""

import sys

import numpy as np

if "/opt/trn_rl_repo" not in sys.path:
    sys.path.insert(0, "/opt/trn_rl_repo")

B, S, D, H = 4, 1024, 1024, 16
HD = D // H                      # 64
SCALE = 1.0 / float(np.sqrt(HD))
N_CORES = 8
SQ = S // 2                      # 512 query rows per core
SK = S                           # full key length
P = 128
NT = D // P                      # 8 feature tiles
SKT = SK // P                    # 8 key tiles
NPAIR = H // 2                   # 8 head pairs
NC2 = 512                        # max matmul free dim (one PSUM bank)

_CACHE = {}


def _build_program():
    from contextlib import ExitStack

    import concourse.bass as bass
    import concourse.tile as tile
    from concourse import bacc, mybir

    F32 = mybir.dt.float32
    F16 = mybir.dt.float16
    AF = mybir.ActivationFunctionType

    nc = bacc.Bacc(
        "TRN2", target_bir_lowering=False, debug=False, num_devices=N_CORES
    )

    qT_d = nc.dram_tensor("qT", [P, NT, SQ], F16, kind="ExternalInput").ap()
    kT_d = nc.dram_tensor("kT", [P, NT, SK], F16, kind="ExternalInput").ap()
    vT_d = nc.dram_tensor("vT", [P, NT, SK], F16, kind="ExternalInput").ap()
    wqT_d = nc.dram_tensor("wqT", [NT, P, NT, P], F16,
                           kind="ExternalInput").ap()
    wkT_d = nc.dram_tensor("wkT", [NT, P, NT, P], F16,
                           kind="ExternalInput").ap()
    wvT_d = nc.dram_tensor("wvT", [2, P, NT, NC2], F16,
                           kind="ExternalInput").ap()
    woT_d = nc.dram_tensor("woT", [P, NT, D], F16, kind="ExternalInput").ap()
    bq_d = nc.dram_tensor("bq", [D], F32, kind="ExternalInput").ap()
    bo_d = nc.dram_tensor("bo", [D], F32, kind="ExternalInput").ap()
    out_d = nc.dram_tensor("out", [SQ, D], F32, kind="ExternalOutput").ap()

    mm = lambda *a, **k: nc.tensor.matmul(*a, **k)

    with tile.TileContext(nc) as tc, ExitStack() as ctx:
        persist = ctx.enter_context(tc.tile_pool(name="persist", bufs=1))
        epool = ctx.enter_context(tc.tile_pool(name="epool", bufs=4))
        rpool = ctx.enter_context(tc.tile_pool(name="rp", bufs=2))
        opool = ctx.enter_context(tc.tile_pool(name="outp", bufs=2))
        pp = ctx.enter_context(tc.tile_pool(name="pp", space="PSUM", bufs=2))
        pS = ctx.enter_context(tc.tile_pool(name="pS", space="PSUM", bufs=2))
        pX = ctx.enter_context(tc.tile_pool(name="pX", space="PSUM", bufs=1))

        # persistent data tiles
        qT_sb = persist.tile([P, NT, SQ], F16)
        kT_sb = persist.tile([P, NT, SK], F16)
        vT_sb = persist.tile([P, NT, SK], F16)
        wq = persist.tile([P, NT, D], F16)
        wk = persist.tile([P, NT, D], F16)
        wv = persist.tile([P, NT, D], F16)
        wo = persist.tile([P, NT, D], F16)
        qhT = persist.tile([P, NT, SQ], F16)        # [o'%128, o'//128, sq]
        khT = persist.tile([P, NT, SK], F16)
        vh = persist.tile([P, SKT, H, HD + 1], F16)  # [sk%128, sk//128, h, .]
        ctxT = persist.tile([P, NT, SQ], F16)
        ctxU = persist.tile([P, NT, SQ], F16)    # unnormalized ctx (drain)
        bq_sb = persist.tile([P, NT], F32)
        bo_bc = persist.tile([P, D], F32)

        # input DMAs ordered by first use. The pair-0 attention critical path
        # is kT -> wk-m0 -> qT -> wq-m0 (scores), then vT/wv (ctx via the
        # just-in-time a-group drains), then the rest. Weight blocks come
        # host-pre-arranged as [m, p, kk, c] so every DMA reads 2KB lines.
        def load_wblock(w_sb, w_blk_d, m):
            nc.sync.dma_start(w_sb[:, :, m * P:(m + 1) * P], w_blk_d[m])

        load_wblock(wk, wkT_d, 0)
        nc.scalar.dma_start(wq[:, :, 0:P], wqT_d[0])
        for kk in range(NT):
            nc.sync.dma_start(kT_sb[:, kk, :], kT_d[:, kk, :])
            nc.scalar.dma_start(qT_sb[:, kk, :], qT_d[:, kk, :])
        nc.sync.dma_start(vT_sb, vT_d)
        nc.scalar.dma_start(wv[:, :, 0:NC2], wvT_d[0])
        for m in range(1, 4):
            load_wblock(wk, wkT_d, m)
            nc.scalar.dma_start(wq[:, :, m * P:(m + 1) * P], wqT_d[m])
        nc.sync.dma_start(wv[:, :, NC2:D], wvT_d[1])
        for m in range(4, NT):
            load_wblock(wk, wkT_d, m)
            load_wblock(wq, wqT_d, m)
        nc.sync.dma_start(wo, woT_d)
        nc.gpsimd.dma_start(out=bq_sb, in_=bq_d.rearrange("(m p) -> p m", p=P))
        bo_bcast_ap = bass.AP(
            tensor=bo_d.tensor, offset=bo_d.offset, ap=[[0, P]] + list(bo_d.ap)
        )
        nc.gpsimd.dma_start(out=bo_bc, in_=bo_bcast_ap)
        for j in range(SKT):
            nc.vector.memset(vh[:, j, :, HD].bitcast(mybir.dt.uint16), 0x3C00)
        warm = rpool.tile([1, 1], F32, name="warm")
        nc.vector.memset(warm, 0.0)
        nc.scalar.activation(warm, warm, AF.Exp)
        ones_sb = persist.tile([1, P], F16)
        nc.vector.memset(ones_sb, 1.0)

        # ---- emit-group helpers (each = one 8-MM PSUM accumulation) ----
        def a_group(j, c):  # v-proj: vh[:, j, heads c*8..c*8+7]
            psa = pp.tile([P, NC2], F32, name="ppt")
            for kk in range(NT):
                mm(psa, vT_sb[:, kk, j * P:(j + 1) * P],
                   wv[:, kk, c * NC2:(c + 1) * NC2],
                   start=kk == 0, stop=kk == NT - 1)
            nc.vector.tensor_copy(
                vh[:, j, c * 8:(c + 1) * 8, 0:HD],
                psa.rearrange("p (h d) -> p h d", d=HD),
            )

        def b_group(m, c):  # k-proj: khT[:, m, c*512:...]
            psb = pp.tile([P, NC2], F32, name="ppt")
            for kk in range(NT):
                mm(psb, wk[:, kk, m * P:(m + 1) * P],
                   kT_sb[:, kk, c * NC2:(c + 1) * NC2],
                   start=kk == 0, stop=kk == NT - 1)
            nc.vector.tensor_copy(khT[:, m, c * NC2:(c + 1) * NC2], psb)

        def c_group(m):  # q-proj: qhT[:, m, :]
            psc = pp.tile([P, NC2], F32, name="ppt")
            for kk in range(NT):
                mm(psc, wq[:, kk, m * P:(m + 1) * P], qT_sb[:, kk, :],
                   start=kk == 0, stop=kk == NT - 1)
            nc.vector.tensor_scalar_add(qhT[:, m, :], psc, bq_sb[:, m:m + 1])

        def e_mms(pse, sqt, c, kks):
            for kk in kks:
                mm(pse, ctxT[:, kk, sqt * P:(sqt + 1) * P],
                   wo[:, kk, c * NC2:(c + 1) * NC2],
                   start=kk == 0, stop=kk == NT - 1)

        def e_finish(pse, sqt, c):
            o_sb = opool.tile([P, NC2], F32, name="o_sb")
            nc.vector.tensor_add(o_sb, pse, bo_bc[:, c * NC2:(c + 1) * NC2])
            nc.sync.dma_start(
                out_d[sqt * P:(sqt + 1) * P, c * NC2:(c + 1) * NC2], o_sb
            )

        def e_group(sqt, c):  # out-proj: out rows sqt*128, cols c*512
            pse = pp.tile([P, NC2], F32, name="ppt")
            e_mms(pse, sqt, c, range(NT))
            e_finish(pse, sqt, c)

        # ---- filler stream with need-driven drains ----
        filler = []          # ordered list of (label, emit_fn)
        emitted = set()

        def drain_until(labels):
            todo = [x for x in labels if x not in emitted]
            if not todo:
                return
            for lbl, fn in filler:
                if lbl not in emitted:
                    emitted.add(lbl)
                    fn()
                if all(x in emitted for x in todo):
                    return

        def drain_next(n=1):
            done = 0
            for lbl, fn in filler:
                if lbl not in emitted:
                    emitted.add(lbl)
                    fn()
                    done += 1
                    if done >= n:
                        return

        # ---- attention ----
        def scores(t, j):
            sp = pS.tile([P, 2, SQ], F32, name="sp")
            mm(sp[:, 0, :], khT[0:HD, t, j * P:(j + 1) * P], qhT[0:HD, t, :],
               start=True, stop=True)
            mm(sp[:, 1, :], khT[HD:P, t, j * P:(j + 1) * P], qhT[HD:P, t, :],
               start=True, stop=True)
            return sp

        # (attention emission is a flat pipelined loop further below)

        def normalize(t, r0, r1):
            rb0 = rpool.tile([P, SQ], F32, name="rb0")
            rb1 = rpool.tile([P, SQ], F32, name="rb1")
            nc.gpsimd.partition_broadcast(rb0, r0)
            nc.gpsimd.partition_broadcast(rb1, r1)
            nc.vector.tensor_mul(ctxT[0:HD, t, :], ctxU[0:HD, t, :],
                                 rb0[0:HD, :])
            nc.vector.tensor_mul(ctxT[HD:P, t, :], ctxU[HD:P, t, :],
                                 rb1[HD:P, :])

        # ---- emission schedule ----
        b_group(0, 0)
        b_group(0, 1)
        c_group(0)

        for j in range(SKT):
            filler.append((f"a{j}c0", lambda j=j: a_group(j, 0)))
        for m in range(1, 4):
            filler.append((f"b{m}a", lambda m=m: b_group(m, 0)))
            filler.append((f"b{m}b", lambda m=m: b_group(m, 1)))
            filler.append((f"c{m}", lambda m=m: c_group(m)))
        for j in range(SKT):
            filler.append((f"a{j}c1", lambda j=j: a_group(j, 1)))
        for m in range(4, NT):
            filler.append((f"b{m}a", lambda m=m: b_group(m, 0)))
            filler.append((f"b{m}b", lambda m=m: b_group(m, 1)))
            filler.append((f"c{m}", lambda m=m: c_group(m)))

        # flat (t, j) pipeline, scores emitted 2 steps ahead so neither PE
        # nor ACT bubbles at pair boundaries
        steps = [(t, j) for t in range(NPAIR) for j in range(SKT)]
        sps = {}

        def emit_scores(idx):
            if idx >= len(steps):
                return
            t, j = steps[idx]
            if j == 0 and t >= 1:
                drain_until([f"b{t}a", f"b{t}b", f"c{t}"])
            sps[idx] = scores(t, j)

        pcx = {}
        rs = {}
        emit_scores(0)
        emit_scores(1)
        for idx, (t, j) in enumerate(steps):
            ep = epool.tile([P, 2, SQ], F16, name="ep")
            nc.scalar.activation(ep, sps.pop(idx), AF.Exp)
            emit_scores(idx + 2)
            drain_until([f"a{j}c{t // 4}"])
            drain_next(1)
            if j == 0:
                pcx[t] = (
                    pX.tile([HD + 1, SQ], F32, name="pcx0"),
                    pX.tile([HD + 1, SQ], F32, name="pcx1"),
                )
            pcx0, pcx1 = pcx[t]
            mm(pcx0, vh[:, j, 2 * t, :], ep[:, 0, :],
               start=j == 0, stop=j == SKT - 1)
            mm(pcx1, vh[:, j, 2 * t + 1, :], ep[:, 1, :],
               start=j == 0, stop=j == SKT - 1)
            if j == SKT - 1:
                # fast PSUM drain: sum-row copies + approx reciprocals gate
                # the (deferred) normalize, so they go first on DVE
                se0 = rpool.tile([1, SQ], F32, name="se0")
                se1 = rpool.tile([1, SQ], F32, name="se1")
                nc.vector.tensor_copy(se0, pcx0[HD:HD + 1, :])
                nc.vector.tensor_copy(se1, pcx1[HD:HD + 1, :])
                r0 = rpool.tile([1, SQ], F32, name="r0")
                r1 = rpool.tile([1, SQ], F32, name="r1")
                nc.vector.reciprocal_approx_fast(r0, se0)
                nc.vector.reciprocal_approx_fast(r1, se1)
                if t == NPAIR - 1:
                    r0h = rpool.tile([1, SQ], F16, name="r0h")
                    r1h = rpool.tile([1, SQ], F16, name="r1h")
                    nc.vector.tensor_copy(r0h, r0)
                    nc.vector.tensor_copy(r1h, r1)
                    rs[t] = (r0h, r1h)
                else:
                    rs[t] = (r0, r1)
                nc.vector.tensor_copy(ctxU[0:HD, t, :], pcx0[0:HD, :])
                nc.vector.tensor_copy(ctxU[HD:P, t, :], pcx1[0:HD, :])
            if j == 2 and t >= 1:
                normalize(t - 1, *rs.pop(t - 1))
            if t == NPAIR - 1 and j == 4:
                psE0 = pp.tile([P, NC2], F32, name="ppt")
                e_mms(psE0, 0, 0, range(NT - 1))
            if t == NPAIR - 1 and j == 6:
                psE1 = pp.tile([P, NC2], F32, name="ppt")
                e_mms(psE1, 0, 1, range(NT - 1))
        # last pair: broadcast the reciprocals on the PE (ones-row matmul
        # into a free scores-pool bank) so E's last inputs arrive fast
        t7 = NPAIR - 1
        r0, r1 = rs.pop(t7)
        rb_ps = pS.tile([P, 2, SQ], F32, name="sp")
        mm(rb_ps[:, 0, :], ones_sb, r0, start=True, stop=True)
        mm(rb_ps[:, 1, :], ones_sb, r1, start=True, stop=True)
        nc.vector.tensor_mul(ctxT[0:HD, t7, :], ctxU[0:HD, t7, :],
                             rb_ps[0:HD, 0, :])
        nc.vector.tensor_mul(ctxT[HD:P, t7, :], ctxU[HD:P, t7, :],
                             rb_ps[HD:P, 1, :])

        drain_until([lbl for lbl, _ in filler])

        # ---- output projection (first two groups already accumulated) ----
        e_mms(psE0, 0, 0, [NT - 1])
        e_finish(psE0, 0, 0)
        e_mms(psE1, 0, 1, [NT - 1])
        e_finish(psE1, 0, 1)
        for sqt in range(1, SQ // P):
            for c in range(2):
                e_group(sqt, c)

    nc.compile()
    return nc


def get_program():
    if "nc" not in _CACHE:
        _CACHE["nc"] = _build_program()
    return _CACHE["nc"]


def make_in_maps(q, k, v, Wq, bq, Wk, bk, Wv, bv, Wo, bo):
    f32 = lambda x: np.ascontiguousarray(np.asarray(x, dtype=np.float32))
    h = lambda x: np.ascontiguousarray(np.asarray(x, dtype=np.float16))
    blk = lambda wT: np.ascontiguousarray(
        np.asarray(wT, np.float16).reshape(NT, P, NT, P).transpose(2, 1, 0, 3)
    )
    # partition-major [p, kk, w]: per-partition data is one contiguous run,
    # so each DMA descriptor covers a full 16KB row (8x fewer descriptors)
    pmaj = lambda xT: np.ascontiguousarray(
        np.asarray(xT, np.float16).reshape(NT, P, -1).transpose(1, 0, 2)
    )
    q, k, v = np.asarray(q, np.float32), np.asarray(k, np.float32), \
        np.asarray(v, np.float32)
    wqT = blk(np.asarray(Wq, np.float32).T * np.float32(SCALE))
    wkT = blk(np.asarray(Wk, np.float32).T)
    wvT_pm = pmaj(np.asarray(Wv, np.float32).T)       # [P, NT, D]
    wvT = np.ascontiguousarray(
        wvT_pm.reshape(P, NT, 2, NC2).transpose(2, 0, 1, 3)
    )                                                 # [2, P, NT, NC2]
    woT = pmaj(np.asarray(Wo, np.float32).T)
    bqs = f32(bq) * np.float32(SCALE)
    # bv folds exactly through the output projection: softmax rows sum to 1,
    # so ctx gains +bv per head, and out gains +Wo@bv
    bo_ = f32(bo) + np.asarray(Wo, np.float32) @ f32(bv)
    # bk is exactly irrelevant: it shifts every score in a query row equally.
    kTs = [pmaj(k[b].T) for b in range(B)]
    vTs = [pmaj(v[b].T) for b in range(B)]
    in_maps = []
    for core in range(N_CORES):
        b, half = divmod(core, 2)
        qT_c = pmaj(q[b, half * SQ:(half + 1) * SQ, :].T)
        in_maps.append({
            "qT": qT_c, "kT": kTs[b], "vT": vTs[b],
            "wqT": wqT, "wkT": wkT, "wvT": wvT, "woT": woT,
            "bq": bqs, "bo": bo_,
        })
    return in_maps


def gather_out(results):
    out = np.empty((B, S, D), dtype=np.float32)
    for core in range(N_CORES):
        b, half = divmod(core, 2)
        out[b, half * SQ:(half + 1) * SQ, :] = results[core]["out"]
    return out


def kernel(q, k, v, Wq, bq, Wk, bk, Wv, bv, Wo, bo):
    from concourse.bass_utils import run_bass_kernel_spmd

    nc = get_program()
    in_maps = make_in_maps(q, k, v, Wq, bq, Wk, bk, Wv, bv, Wo, bo)
    res = run_bass_kernel_spmd(nc, in_maps, list(range(N_CORES)))
    return gather_out(res.results)


# revision 25
# speedup vs baseline: 1.0264x; 1.0264x over previous
"""Trainium2 Bass kernel for nn_Attention (B=4, S=1024, D=1024, H=16).

Sharding: 8 cores = 4 batches x 2 query-halves. Core i handles batch i//2,
query rows [(i%2)*512, (i%2)*512+512). Each core computes the full K/V
projections for its batch (duplicated across the 2 cores sharing a batch),
all 16 heads of attention for its query slice, and the output projection.
No collectives; the output is gathered host-side.

Device dataflow (per core) — fp16 matmul operands, fp32 PSUM accumulation:
  - host passes pre-transposed qT [D,SQ], kT/vT [D,SK], W{v,o}.T [D,D], and
    m-blocked W{k,q}.T [NT,D,128] (PE contracts over the partition dim, so
    both matmul operands need the contraction dim on partitions; transposing
    and blocking on host costs nothing on HW)
  - qhT[o,sq] = (Wq.T*SCALE).T-tiles @ qT     (o on partitions)
  - khT[o,sk] likewise; vh[sk, h, dh] natural via vT-as-stationary
  - scoresT[sk,sq] per head = khT-tile.T @ qhT; the two heads of a pair run
    as K=64 matmuls packed at PE row strips 0:64 / 64:128, writing the two
    halves of one [128, 2*SQ] PSUM tile -> ONE fused exp per pair
  - expT = exp(scoresT) on ACT (no max subtraction: |scores| < ~4 here,
    and softmax(x) == softmax(x - max) exactly)
  - ctxT_aug[dh+1, sq] += [vh | 1].T @ expT  (ones column makes row 64 the
    softmax denominator, riding free on the ctx matmul)
  - ctx PSUM is drained fast (ACT copies + approx-reciprocal of the sum
    row); the normalization multiply runs later, off the critical path
  - out[sq,o] = ctxT-tiles.T @ Wo.T + bo      (natural layout -> direct DMA)

The attention phase is ACT(exp)-bound per-step, so projection-matmul
"filler" groups are interleaved into the attention emission order (with
need-driven draining) to keep the PE busy and HAM-warm throughout.

Bias handling (exact): bq via per-partition add on the qh copy; bk dropped
(softmax is invariant to per-query score shifts); bv added after normalize
(softmax rows sum to 1); bo via a partition-broadcast tile.
"""

import sys

import numpy as np

if "/opt/trn_rl_repo" not in sys.path:
    sys.path.insert(0, "/opt/trn_rl_repo")

B, S, D, H = 4, 1024, 1024, 16
HD = D // H                      # 64
SCALE = 1.0 / float(np.sqrt(HD))
N_CORES = 8
SQ = S // 2                      # 512 query rows per core
SK = S                           # full key length
P = 128
NT = D // P                      # 8 feature tiles
SKT = SK // P                    # 8 key tiles
NPAIR = H // 2                   # 8 head pairs
NC2 = 512                        # max matmul free dim (one PSUM bank)

_CACHE = {}


def _build_program():
    from contextlib import ExitStack

    import concourse.bass as bass
    import concourse.tile as tile
    from concourse import bacc, mybir

    F32 = mybir.dt.float32
    F16 = mybir.dt.float16
    AF = mybir.ActivationFunctionType

    nc = bacc.Bacc(
        "TRN2", target_bir_lowering=False, debug=False, num_devices=N_CORES
    )

    qT_d = nc.dram_tensor("qT", [P, NT, SQ], F16, kind="ExternalInput").ap()
    kT_d = nc.dram_tensor("kT", [P, NT, SK], F16, kind="ExternalInput").ap()
    vT_d = nc.dram_tensor("vT", [P, NT, SK], F16, kind="ExternalInput").ap()
    wqT_d = nc.dram_tensor("wqT", [NT, P, NT, P], F16,
                           kind="ExternalInput").ap()
    wkT_d = nc.dram_tensor("wkT", [NT, P, NT, P], F16,
                           kind="ExternalInput").ap()
    wvT_d = nc.dram_tensor("wvT", [2, P, NT, NC2], F16,
                           kind="ExternalInput").ap()
    woT_d = nc.dram_tensor("woT", [P, NT, D], F16, kind="ExternalInput").ap()
    bq_d = nc.dram_tensor("bq", [D], F32, kind="ExternalInput").ap()
    bo_d = nc.dram_tensor("bo", [D], F32, kind="ExternalInput").ap()
    out_d = nc.dram_tensor("out", [SQ, D], F32, kind="ExternalOutput").ap()

    mm = lambda *a, **k: nc.tensor.matmul(*a, **k)

    with tile.TileContext(nc) as tc, ExitStack() as ctx:
        persist = ctx.enter_context(tc.tile_pool(name="persist", bufs=1))
        epool = ctx.enter_context(tc.tile_pool(name="epool", bufs=4))
        rpool = ctx.enter_context(tc.tile_pool(name="rp", bufs=2))
        opool = ctx.enter_context(tc.tile_pool(name="outp", bufs=2))
        pp = ctx.enter_context(tc.tile_pool(name="pp", space="PSUM", bufs=2))
        pS = ctx.enter_context(tc.tile_pool(name="pS", space="PSUM", bufs=2))
        pX = ctx.enter_context(tc.tile_pool(name="pX", space="PSUM", bufs=1))

        # persistent data tiles
        qT_sb = persist.tile([P, NT, SQ], F16)
        kT_sb = persist.tile([P, NT, SK], F16)
        vT_sb = persist.tile([P, NT, SK], F16)
        wq = persist.tile([P, NT, D], F16)
        wk = persist.tile([P, NT, D], F16)
        wv = persist.tile([P, NT, D], F16)
        wo = persist.tile([P, NT, D], F16)
        qhT = persist.tile([P, NT, SQ], F16)        # [o'%128, o'//128, sq]
        khT = persist.tile([P, NT, SK], F16)
        vh = persist.tile([P, SKT, H, HD + 1], F16)  # [sk%128, sk//128, h, .]
        ctxT = persist.tile([P, NT, SQ], F16)
        ctxU = persist.tile([P, NT, SQ], F16)    # unnormalized ctx (drain)
        bq_sb = persist.tile([P, NT], F32)
        bo_bc = persist.tile([P, D], F32)

        # input DMAs ordered by first use. The pair-0 attention critical path
        # is kT -> wk-m0 -> qT -> wq-m0 (scores), then vT/wv (ctx via the
        # just-in-time a-group drains), then the rest. Weight blocks come
        # host-pre-arranged as [m, p, kk, c] so every DMA reads 2KB lines.
        def load_wblock(w_sb, w_blk_d, m):
            nc.sync.dma_start(w_sb[:, :, m * P:(m + 1) * P], w_blk_d[m])

        load_wblock(wk, wkT_d, 0)
        nc.scalar.dma_start(wq[:, :, 0:P], wqT_d[0])
        for kk in range(NT):
            nc.sync.dma_start(kT_sb[:, kk, :], kT_d[:, kk, :])
            nc.scalar.dma_start(qT_sb[:, kk, :], qT_d[:, kk, :])
        nc.sync.dma_start(vT_sb, vT_d)
        nc.scalar.dma_start(wv[:, :, 0:NC2], wvT_d[0])
        for m in range(1, 4):
            load_wblock(wk, wkT_d, m)
            nc.scalar.dma_start(wq[:, :, m * P:(m + 1) * P], wqT_d[m])
        nc.sync.dma_start(wv[:, :, NC2:D], wvT_d[1])
        for m in range(4, NT):
            load_wblock(wk, wkT_d, m)
            load_wblock(wq, wqT_d, m)
        nc.sync.dma_start(wo, woT_d)
        nc.gpsimd.dma_start(out=bq_sb, in_=bq_d.rearrange("(m p) -> p m", p=P))
        bo_bcast_ap = bass.AP(
            tensor=bo_d.tensor, offset=bo_d.offset, ap=[[0, P]] + list(bo_d.ap)
        )
        nc.gpsimd.dma_start(out=bo_bc, in_=bo_bcast_ap)
        for j in range(SKT):
            nc.vector.memset(vh[:, j, :, HD].bitcast(mybir.dt.uint16), 0x3C00)
        warm = rpool.tile([1, 1], F32, name="warm")
        nc.vector.memset(warm, 0.0)
        nc.scalar.activation(warm, warm, AF.Exp)
        # dummy matmuls on a zeroed tile during the DMA ramp: HAM sees a busy
        # PE and unthrottles to 2.4 GHz before the real matmuls start
        wz = persist.tile([P, NC2], F16)
        nc.vector.memset(wz, 0.0)
        psw = pp.tile([P, NC2], F32, name="ppt")
        for _ in range(48):
            mm(psw, wz[:, 0:P], wz, start=True, stop=True)
        ones_sb = persist.tile([1, P], F16)
        nc.vector.memset(ones_sb, 1.0)

        # ---- emit-group helpers (each = one 8-MM PSUM accumulation) ----
        def a_group(j, c):  # v-proj: vh[:, j, heads c*8..c*8+7]
            psa = pp.tile([P, NC2], F32, name="ppt")
            for kk in range(NT):
                mm(psa, vT_sb[:, kk, j * P:(j + 1) * P],
                   wv[:, kk, c * NC2:(c + 1) * NC2],
                   start=kk == 0, stop=kk == NT - 1)
            nc.vector.tensor_copy(
                vh[:, j, c * 8:(c + 1) * 8, 0:HD],
                psa.rearrange("p (h d) -> p h d", d=HD),
            )

        def b_group(m, c):  # k-proj: khT[:, m, c*512:...]
            psb = pp.tile([P, NC2], F32, name="ppt")
            for kk in range(NT):
                mm(psb, wk[:, kk, m * P:(m + 1) * P],
                   kT_sb[:, kk, c * NC2:(c + 1) * NC2],
                   start=kk == 0, stop=kk == NT - 1)
            nc.vector.tensor_copy(khT[:, m, c * NC2:(c + 1) * NC2], psb)

        def c_group(m):  # q-proj: qhT[:, m, :]
            psc = pp.tile([P, NC2], F32, name="ppt")
            for kk in range(NT):
                mm(psc, wq[:, kk, m * P:(m + 1) * P], qT_sb[:, kk, :],
                   start=kk == 0, stop=kk == NT - 1)
            nc.vector.tensor_scalar_add(qhT[:, m, :], psc, bq_sb[:, m:m + 1])

        def e_mms(pse, sqt, c, kks):
            for kk in kks:
                mm(pse, ctxT[:, kk, sqt * P:(sqt + 1) * P],
                   wo[:, kk, c * NC2:(c + 1) * NC2],
                   start=kk == 0, stop=kk == NT - 1)

        def e_finish(pse, sqt, c):
            o_sb = opool.tile([P, NC2], F32, name="o_sb")
            nc.vector.tensor_add(o_sb, pse, bo_bc[:, c * NC2:(c + 1) * NC2])
            nc.sync.dma_start(
                out_d[sqt * P:(sqt + 1) * P, c * NC2:(c + 1) * NC2], o_sb
            )

        def e_group(sqt, c):  # out-proj: out rows sqt*128, cols c*512
            pse = pp.tile([P, NC2], F32, name="ppt")
            e_mms(pse, sqt, c, range(NT))
            e_finish(pse, sqt, c)

        # ---- filler stream with need-driven drains ----
        filler = []          # ordered list of (label, emit_fn)
        emitted = set()

        def drain_until(labels):
            todo = [x for x in labels if x not in emitted]
            if not todo:
                return
            for lbl, fn in filler:
                if lbl not in emitted:
                    emitted.add(lbl)
                    fn()
                if all(x in emitted for x in todo):
                    return

        def drain_next(n=1):
            done = 0
            for lbl, fn in filler:
                if lbl not in emitted:
                    emitted.add(lbl)
                    fn()
                    done += 1
                    if done >= n:
                        return

        # ---- attention ----
        def scores(t, j):
            sp = pS.tile([P, 2, SQ], F32, name="sp")
            mm(sp[:, 0, :], khT[0:HD, t, j * P:(j + 1) * P], qhT[0:HD, t, :],
               start=True, stop=True)
            mm(sp[:, 1, :], khT[HD:P, t, j * P:(j + 1) * P], qhT[HD:P, t, :],
               start=True, stop=True)
            return sp

        # (attention emission is a flat pipelined loop further below)

        def normalize(t, r0, r1):
            rb0 = rpool.tile([P, SQ], F32, name="rb0")
            rb1 = rpool.tile([P, SQ], F32, name="rb1")
            nc.gpsimd.partition_broadcast(rb0, r0)
            nc.gpsimd.partition_broadcast(rb1, r1)
            nc.vector.tensor_mul(ctxT[0:HD, t, :], ctxU[0:HD, t, :],
                                 rb0[0:HD, :])
            nc.vector.tensor_mul(ctxT[HD:P, t, :], ctxU[HD:P, t, :],
                                 rb1[HD:P, :])

        # ---- emission schedule ----
        b_group(0, 0)
        b_group(0, 1)
        c_group(0)

        for j in range(SKT):
            filler.append((f"a{j}c0", lambda j=j: a_group(j, 0)))
        for m in range(1, 4):
            filler.append((f"b{m}a", lambda m=m: b_group(m, 0)))
            filler.append((f"b{m}b", lambda m=m: b_group(m, 1)))
            filler.append((f"c{m}", lambda m=m: c_group(m)))
        for j in range(SKT):
            filler.append((f"a{j}c1", lambda j=j: a_group(j, 1)))
        for m in range(4, NT):
            filler.append((f"b{m}a", lambda m=m: b_group(m, 0)))
            filler.append((f"b{m}b", lambda m=m: b_group(m, 1)))
            filler.append((f"c{m}", lambda m=m: c_group(m)))

        # flat (t, j) pipeline, scores emitted 2 steps ahead so neither PE
        # nor ACT bubbles at pair boundaries
        steps = [(t, j) for t in range(NPAIR) for j in range(SKT)]
        sps = {}

        def emit_scores(idx):
            if idx >= len(steps):
                return
            t, j = steps[idx]
            if j == 0 and t >= 1:
                drain_until([f"b{t}a", f"b{t}b", f"c{t}"])
            sps[idx] = scores(t, j)

        pcx = {}
        rs = {}
        emit_scores(0)
        emit_scores(1)
        for idx, (t, j) in enumerate(steps):
            ep = epool.tile([P, 2, SQ], F16, name="ep")
            nc.scalar.activation(ep, sps.pop(idx), AF.Exp)
            emit_scores(idx + 2)
            drain_until([f"a{j}c{t // 4}"])
            drain_next(1)
            if j == 0:
                pcx[t] = (
                    pX.tile([HD + 1, SQ], F32, name="pcx0"),
                    pX.tile([HD + 1, SQ], F32, name="pcx1"),
                )
            pcx0, pcx1 = pcx[t]
            mm(pcx0, vh[:, j, 2 * t, :], ep[:, 0, :],
               start=j == 0, stop=j == SKT - 1)
            mm(pcx1, vh[:, j, 2 * t + 1, :], ep[:, 1, :],
               start=j == 0, stop=j == SKT - 1)
            if j == SKT - 1:
                # fast PSUM drain: sum-row copies + approx reciprocals gate
                # the (deferred) normalize, so they go first on DVE
                se0 = rpool.tile([1, SQ], F32, name="se0")
                se1 = rpool.tile([1, SQ], F32, name="se1")
                nc.vector.tensor_copy(se0, pcx0[HD:HD + 1, :])
                nc.vector.tensor_copy(se1, pcx1[HD:HD + 1, :])
                r0 = rpool.tile([1, SQ], F32, name="r0")
                r1 = rpool.tile([1, SQ], F32, name="r1")
                nc.vector.reciprocal_approx_fast(r0, se0)
                nc.vector.reciprocal_approx_fast(r1, se1)
                if t == NPAIR - 1:
                    r0h = rpool.tile([1, SQ], F16, name="r0h")
                    r1h = rpool.tile([1, SQ], F16, name="r1h")
                    nc.vector.tensor_copy(r0h, r0)
                    nc.vector.tensor_copy(r1h, r1)
                    rs[t] = (r0h, r1h)
                else:
                    rs[t] = (r0, r1)
                nc.vector.tensor_copy(ctxU[0:HD, t, :], pcx0[0:HD, :])
                nc.vector.tensor_copy(ctxU[HD:P, t, :], pcx1[0:HD, :])
            if j == 2 and t >= 1:
                normalize(t - 1, *rs.pop(t - 1))
            if t == NPAIR - 1 and j == 4:
                psE0 = pp.tile([P, NC2], F32, name="ppt")
                e_mms(psE0, 0, 0, range(NT - 1))
            if t == NPAIR - 1 and j == 6:
                psE1 = pp.tile([P, NC2], F32, name="ppt")
                e_mms(psE1, 0, 1, range(NT - 1))
        # last pair: broadcast the reciprocals on the PE (ones-row matmul
        # into a free scores-pool bank) so E's last inputs arrive fast
        t7 = NPAIR - 1
        r0, r1 = rs.pop(t7)
        rb_ps = pS.tile([P, 2, SQ], F32, name="sp")
        mm(rb_ps[:, 0, :], ones_sb, r0, start=True, stop=True)
        mm(rb_ps[:, 1, :], ones_sb, r1, start=True, stop=True)
        nc.vector.tensor_mul(ctxT[0:HD, t7, :], ctxU[0:HD, t7, :],
                             rb_ps[0:HD, 0, :])
        nc.vector.tensor_mul(ctxT[HD:P, t7, :], ctxU[HD:P, t7, :],
                             rb_ps[HD:P, 1, :])

        drain_until([lbl for lbl, _ in filler])

        # ---- output projection (first two groups already accumulated) ----
        e_mms(psE0, 0, 0, [NT - 1])
        e_finish(psE0, 0, 0)
        e_mms(psE1, 0, 1, [NT - 1])
        e_finish(psE1, 0, 1)
        for sqt in range(1, SQ // P):
            for c in range(2):
                e_group(sqt, c)

    nc.compile()
    return nc


def get_program():
    if "nc" not in _CACHE:
        _CACHE["nc"] = _build_program()
    return _CACHE["nc"]


def make_in_maps(q, k, v, Wq, bq, Wk, bk, Wv, bv, Wo, bo):
    f32 = lambda x: np.ascontiguousarray(np.asarray(x, dtype=np.float32))
    h = lambda x: np.ascontiguousarray(np.asarray(x, dtype=np.float16))
    blk = lambda wT: np.ascontiguousarray(
        np.asarray(wT, np.float16).reshape(NT, P, NT, P).transpose(2, 1, 0, 3)
    )
    # partition-major [p, kk, w]: per-partition data is one contiguous run,
    # so each DMA descriptor covers a full 16KB row (8x fewer descriptors)
    pmaj = lambda xT: np.ascontiguousarray(
        np.asarray(xT, np.float16).reshape(NT, P, -1).transpose(1, 0, 2)
    )
    q, k, v = np.asarray(q, np.float32), np.asarray(k, np.float32), \
        np.asarray(v, np.float32)
    wqT = blk(np.asarray(Wq, np.float32).T * np.float32(SCALE))
    wkT = blk(np.asarray(Wk, np.float32).T)
    wvT_pm = pmaj(np.asarray(Wv, np.float32).T)       # [P, NT, D]
    wvT = np.ascontiguousarray(
        wvT_pm.reshape(P, NT, 2, NC2).transpose(2, 0, 1, 3)
    )                                                 # [2, P, NT, NC2]
    woT = pmaj(np.asarray(Wo, np.float32).T)
    bqs = f32(bq) * np.float32(SCALE)
    # bv folds exactly through the output projection: softmax rows sum to 1,
    # so ctx gains +bv per head, and out gains +Wo@bv
    bo_ = f32(bo) + np.asarray(Wo, np.float32) @ f32(bv)
    # bk is exactly irrelevant: it shifts every score in a query row equally.
    kTs = [pmaj(k[b].T) for b in range(B)]
    vTs = [pmaj(v[b].T) for b in range(B)]
    in_maps = []
    for core in range(N_CORES):
        b, half = divmod(core, 2)
        qT_c = pmaj(q[b, half * SQ:(half + 1) * SQ, :].T)
        in_maps.append({
            "qT": qT_c, "kT": kTs[b], "vT": vTs[b],
            "wqT": wqT, "wkT": wkT, "wvT": wvT, "woT": woT,
            "bq": bqs, "bo": bo_,
        })
    return in_maps


def gather_out(results):
    out = np.empty((B, S, D), dtype=np.float32)
    for core in range(N_CORES):
        b, half = divmod(core, 2)
        out[b, half * SQ:(half + 1) * SQ, :] = results[core]["out"]
    return out


def kernel(q, k, v, Wq, bq, Wk, bk, Wv, bv, Wo, bo):
    from concourse.bass_utils import run_bass_kernel_spmd

    nc = get_program()
    in_maps = make_in_maps(q, k, v, Wq, bq, Wk, bk, Wv, bv, Wo, bo)
    res = run_bass_kernel_spmd(nc, in_maps, list(range(N_CORES)))
    return gather_out(res.results)
